# revision 1
# baseline (speedup 1.0000x reference)
"""DAM-Softmax loss kernel for Trainium2 (Bass/Tile), 8-core data parallel.

Math (per sample b, target t = label[b]):
    cos_t  = costh[b, t]
    delta  = (MARGIN/LAMDA) * exp(1 - cos_t)
    logits = S * costh, with logits[b, t] = S * (cos_t - delta)
    loss_b = logsumexp(logits[b, :]) - S * (cos_t - delta)
    loss   = mean_b loss_b

Since costh is bounded in [0, 1), we use the constant M = 1.0 as the
stability shift (exp arguments stay in [-S, 0]) instead of a per-row max:
    ssum   = sum_j exp(S*(costh[b,j] - M))
    Z      = ssum - exp(S*(cos_t - M)) + exp(S*(cos_t - delta - M))
    loss_b = S*M + ln(Z) - S*(cos_t - delta)

Sharding: batch dim split evenly across 8 NeuronCores (data parallel).
Each core streams its [1024, 10000] f32 shard from HBM once (memory-bound),
computes per-sample losses on device, and the host mean-reduces the
8 x [128, 8] per-sample loss outputs.
"""

import numpy as np

NCORES = 8
B, C = 8192, 10000
R = B // NCORES          # rows per core
P = 128                  # SBUF partitions
T = R // P               # row tiles per core
NCH = 2                  # column chunks per row tile
W = C // NCH             # chunk width
S = 15.0
MARGIN = 0.3
LAMDA = 2.0
DCOEF = MARGIN / LAMDA
MAXC = 1.0               # upper bound of costh (uniform [0,1)) used as exp shift

_NC_CACHE = {}


def _build_nc(big_bufs=16, repeat=1, nch=8):
    # repeat > 1 re-streams the shard `repeat` times inside one NEFF; used by
    # the timing harness to infer per-pass device time from the wall-clock
    # slope (axon dispatch overhead cancels in the difference).
    import concourse.bacc as bacc
    import concourse.bass as bass
    import concourse.mybir as mybir
    import concourse.tile as tile

    f32 = mybir.dt.float32
    i32 = mybir.dt.int32
    Act = mybir.ActivationFunctionType
    Alu = mybir.AluOpType

    nc = bacc.Bacc(None, target_bir_lowering=False, debug=False)

    costh = nc.dram_tensor("costh", [R, C], f32, kind="ExternalInput")
    label = nc.dram_tensor("label", [R], i32, kind="ExternalInput")
    out = nc.dram_tensor("out", [P, T], f32, kind="ExternalOutput")

    with tile.TileContext(nc) as tc:
        with (
            tc.tile_pool(name="big", bufs=big_bufs) as big,
            tc.tile_pool(name="small", bufs=1) as small,
        ):
            # bias vector for exp(S*x - S*M) activations
            neg_sm = small.tile([P, 1], f32)
            nc.vector.memset(neg_sm[:], -S * MAXC)

            # --- prologue: gather target cosines cos_t[p, t] = costh[t*P+p, label] ---
            label_sb = small.tile([P, T], i32)
            nc.gpsimd.dma_start(
                out=label_sb[:], in_=label[:].rearrange("(t p) -> p t", p=P)
            )
            # idx[p, t] = (t*P + p) * C + label  (flat element index), computed
            # in f32 (exact: values < 2^24) since iota steps are limited to i16.
            row_i = small.tile([P, T], i32)
            nc.gpsimd.iota(row_i[:], pattern=[[P, T]], base=0, channel_multiplier=1)
            row_f = small.tile([P, T], f32)
            nc.vector.tensor_copy(out=row_f[:], in_=row_i[:])
            lab_f = small.tile([P, T], f32)
            nc.vector.tensor_copy(out=lab_f[:], in_=label_sb[:])
            idx_f = small.tile([P, T], f32)
            nc.vector.scalar_tensor_tensor(
                out=idx_f[:], in0=row_f[:], scalar=float(C), in1=lab_f[:],
                op0=Alu.mult, op1=Alu.add,
            )
            idx = small.tile([P, T], i32)
            nc.vector.tensor_copy(out=idx[:], in_=idx_f[:])
            # one indirect DMA per column: HW honors only one index per
            # partition per gather (multi-column offset APs misbehave on HW)
            cos_t = small.tile([P, T], f32)
            for t in range(T):
                nc.gpsimd.indirect_dma_start(
                    out=cos_t[:, t:t + 1],
                    out_offset=None,
                    in_=costh[:, :],
                    in_offset=bass.IndirectOffsetOnAxis(ap=idx[:, t:t + 1], axis=1),
                )

            # target-term math depends only on cos_t, so it is emitted before
            # the stream and overlaps it:
            #   delta_e = exp(1 - cos_t);  ct_adj = cos_t - DCOEF * delta_e
            #   e12 = exp(S*(cos_t - M)) - exp(S*(ct_adj - M))
            delta_e = small.tile([P, T], f32)
            nc.scalar.activation(
                out=delta_e[:], in_=cos_t[:], func=Act.Exp, bias=1.0, scale=-1.0
            )
            ct_adj = small.tile([P, T], f32)
            nc.vector.scalar_tensor_tensor(
                out=ct_adj[:], in0=delta_e[:], scalar=-DCOEF, in1=cos_t[:],
                op0=Alu.mult, op1=Alu.add,
            )
            e1 = small.tile([P, T], f32)
            nc.scalar.activation(
                out=e1[:], in_=cos_t[:], func=Act.Exp, bias=neg_sm[:], scale=S
            )
            e2 = small.tile([P, T], f32)
            nc.scalar.activation(
                out=e2[:], in_=ct_adj[:], func=Act.Exp, bias=neg_sm[:], scale=S
            )
            e12 = small.tile([P, T], f32)
            nc.vector.tensor_sub(out=e12[:], in0=e1[:], in1=e2[:])

            # --- main loop: stream shard, fused exp + row-sum on ACT ---
            # per-tile partial reduces overlap the stream; only tile T-1's
            # reduce is on the post-stream critical path
            w = C // nch
            ssum_parts = small.tile([P, T * nch], f32)
            ssums = small.tile([P, T], f32)
            for _rep in range(repeat):
                for t in range(T):
                    for h in range(nch):
                        xc = big.tile([P, w], f32, tag="xc")
                        nc.sync.dma_start(
                            out=xc[:], in_=costh[t * P:(t + 1) * P, h * w:(h + 1) * w]
                        )
                        k = t * nch + h
                        nc.scalar.activation(
                            out=xc[:],
                            in_=xc[:],
                            func=Act.Exp,
                            bias=neg_sm[:],
                            scale=S,
                            accum_out=ssum_parts[:, k:k + 1],
                        )
                    if _rep == repeat - 1:
                        nc.vector.reduce_sum(
                            out=ssums[:, t:t + 1],
                            in_=ssum_parts[:, t * nch:(t + 1) * nch],
                            axis=mybir.AxisListType.X,
                        )

            # --- tail: z = ssums - e12; loss_dev = ln(z) - S*ct_adj ---
            z = small.tile([P, T], f32)
            nc.vector.tensor_sub(out=z[:], in0=ssums[:], in1=e12[:])
            lnz = small.tile([P, T], f32)
            nc.scalar.activation(out=lnz[:], in_=z[:], func=Act.Ln)
            loss = small.tile([P, T], f32)
            nc.vector.scalar_tensor_tensor(
                out=loss[:], in0=ct_adj[:], scalar=-S, in1=lnz[:],
                op0=Alu.mult, op1=Alu.add,
            )
            nc.sync.dma_start(out=out[:], in_=loss[:])

    nc.compile()
    return nc


def _get_nc():
    if "nc" not in _NC_CACHE:
        _NC_CACHE["nc"] = _build_nc()
    return _NC_CACHE["nc"]


def _run(costh_np, label_np, trace=False, **spmd_kwargs):
    from concourse.bass_utils import run_bass_kernel_spmd

    nc = _get_nc()
    costh_np = np.ascontiguousarray(costh_np, dtype=np.float32)
    label_i32 = np.ascontiguousarray(label_np).astype(np.int32)
    in_maps = [
        {
            "costh": costh_np[k * R:(k + 1) * R],
            "label": label_i32[k * R:(k + 1) * R],
        }
        for k in range(NCORES)
    ]
    # The first execution of a fresh NEFF through the axon tunnel
    # occasionally faults with NRT_EXEC_UNIT_UNRECOVERABLE; failures are
    # loud (exception, never silent corruption), so a bounded retry is safe.
    # A non-finite total also triggers a retry as extra insurance.
    last_exc = None
    for _attempt in range(3):
        try:
            res = run_bass_kernel_spmd(
                nc, in_maps, core_ids=list(range(NCORES)), trace=trace,
                **spmd_kwargs
            )
            total = sum(r["out"].astype(np.float64).sum() for r in res.results)
            if np.isfinite(total):
                break
            last_exc = RuntimeError("non-finite loss from device")
        except Exception as exc:  # noqa: BLE001
            last_exc = exc
    else:
        raise last_exc
    loss = np.float32(total / B + S * MAXC)
    return loss, res


def kernel(costh, label):
    loss, _ = _run(costh, label)
    return loss



# revision 2
# speedup vs baseline: 161.9534x; 161.9534x over previous
"""DAM-Softmax loss kernel for Trainium2 (Bass/Tile), 8-core data parallel.

Math (per sample b, target t = label[b]):
    cos_t  = costh[b, t]
    delta  = (MARGIN/LAMDA) * exp(1 - cos_t)
    logits = S * costh, with logits[b, t] = S * (cos_t - delta)
    loss_b = logsumexp(logits[b, :]) - S * (cos_t - delta)
    loss   = mean_b loss_b

Since costh is bounded in [0, 1), we use the constant M = 1.0 as the
stability shift (exp arguments stay in [-S, 0]) instead of a per-row max:
    ssum   = sum_j exp(S*(costh[b,j] - M))
    Z      = ssum - exp(S*(cos_t - M)) + exp(S*(cos_t - delta - M))
    loss_b = S*M + ln(Z) - S*(cos_t - delta)

Sharding: batch dim split evenly across 8 NeuronCores (data parallel).
Each core streams its [1024, 10000] f32 shard from HBM once (memory-bound),
computes per-sample losses on device, and the host mean-reduces the
8 x [128, 8] per-sample loss outputs.

Tuning (measured via the wall-clock slope method, see timing.py):
  - DMA shape: 2 column chunks per [128, 10000] row tile -> 2.56 MB
    transfers with 20 kB contiguous per partition; sustains ~429 GB/s/core
    vs ~389 GB/s at the previous 8-chunk shape.
  - bufs=4 on the streaming pool (deeper made it slower).
  - The per-row exp-sum partials accumulate via ACT accum_out only; the
    DVE reduce over partials runs ONCE in the epilogue instead of per row
    tile (the in-loop reduce cost ~10 us/pass of DMA/ACT interference).
  - Partials are laid out chunk-major [P, NCH*T] so the epilogue combine
    is one contiguous tensor_add, not a strided gather.
"""

import numpy as np

NCORES = 8
B, C = 8192, 10000
R = B // NCORES          # rows per core
P = 128                  # SBUF partitions
T = R // P               # row tiles per core
NCH = 2                  # column chunks per row tile
W = C // NCH             # chunk width
S = 15.0
MARGIN = 0.3
LAMDA = 2.0
DCOEF = MARGIN / LAMDA
MAXC = 1.0               # upper bound of costh (uniform [0,1)) used as exp shift

_NC_CACHE = {}


def _build_nc(big_bufs=4, repeat=1, nch=NCH):
    # repeat > 1 re-streams the shard `repeat` times inside one NEFF; used by
    # the timing harness to infer per-pass device time from the wall-clock
    # slope (axon dispatch overhead cancels in the difference).
    import concourse.bacc as bacc
    import concourse.bass as bass
    import concourse.mybir as mybir
    import concourse.tile as tile

    f32 = mybir.dt.float32
    i32 = mybir.dt.int32
    Act = mybir.ActivationFunctionType
    Alu = mybir.AluOpType

    nc = bacc.Bacc(None, target_bir_lowering=False, debug=False)

    costh = nc.dram_tensor("costh", [R, C], f32, kind="ExternalInput")
    label = nc.dram_tensor("label", [R], i32, kind="ExternalInput")
    out = nc.dram_tensor("out", [P, T], f32, kind="ExternalOutput")

    with tile.TileContext(nc) as tc:
        with (
            tc.tile_pool(name="big", bufs=big_bufs) as big,
            tc.tile_pool(name="small", bufs=1) as small,
        ):
            # bias vector for exp(S*x - S*M) activations
            neg_sm = small.tile([P, 1], f32)
            nc.vector.memset(neg_sm[:], -S * MAXC)

            # --- prologue: gather target cosines cos_t[p, t] = costh[t*P+p, label] ---
            label_sb = small.tile([P, T], i32)
            nc.gpsimd.dma_start(
                out=label_sb[:], in_=label[:].rearrange("(t p) -> p t", p=P)
            )
            # idx[p, t] = (t*P + p) * C + label  (flat element index), computed
            # in f32 (exact: values < 2^24) since iota steps are limited to i16.
            row_i = small.tile([P, T], i32)
            nc.gpsimd.iota(row_i[:], pattern=[[P, T]], base=0, channel_multiplier=1)
            row_f = small.tile([P, T], f32)
            nc.vector.tensor_copy(out=row_f[:], in_=row_i[:])
            lab_f = small.tile([P, T], f32)
            nc.vector.tensor_copy(out=lab_f[:], in_=label_sb[:])
            idx_f = small.tile([P, T], f32)
            nc.vector.scalar_tensor_tensor(
                out=idx_f[:], in0=row_f[:], scalar=float(C), in1=lab_f[:],
                op0=Alu.mult, op1=Alu.add,
            )
            idx = small.tile([P, T], i32)
            nc.vector.tensor_copy(out=idx[:], in_=idx_f[:])
            # one indirect DMA per column: HW honors only one index per
            # partition per gather (multi-column offset APs misbehave on HW)
            cos_t = small.tile([P, T], f32)
            for t in range(T):
                nc.gpsimd.indirect_dma_start(
                    out=cos_t[:, t:t + 1],
                    out_offset=None,
                    in_=costh[:, :],
                    in_offset=bass.IndirectOffsetOnAxis(ap=idx[:, t:t + 1], axis=1),
                )

            # target-term math depends only on cos_t, so it is emitted before
            # the stream and overlaps it:
            #   delta_e = exp(1 - cos_t);  ct_adj = cos_t - DCOEF * delta_e
            #   e12 = exp(S*(cos_t - M)) - exp(S*(ct_adj - M))
            delta_e = small.tile([P, T], f32)
            nc.scalar.activation(
                out=delta_e[:], in_=cos_t[:], func=Act.Exp, bias=1.0, scale=-1.0
            )
            ct_adj = small.tile([P, T], f32)
            nc.vector.scalar_tensor_tensor(
                out=ct_adj[:], in0=delta_e[:], scalar=-DCOEF, in1=cos_t[:],
                op0=Alu.mult, op1=Alu.add,
            )
            e1 = small.tile([P, T], f32)
            nc.scalar.activation(
                out=e1[:], in_=cos_t[:], func=Act.Exp, bias=neg_sm[:], scale=S
            )
            e2 = small.tile([P, T], f32)
            nc.scalar.activation(
                out=e2[:], in_=ct_adj[:], func=Act.Exp, bias=neg_sm[:], scale=S
            )
            e12 = small.tile([P, T], f32)
            nc.vector.tensor_sub(out=e12[:], in0=e1[:], in1=e2[:])

            # --- main loop: stream shard, fused exp + row-sum on ACT ---
            # partials land chunk-major: column h*T + t, so chunk h of all
            # row tiles is one contiguous [P, T] block. No DVE work in the
            # loop — the combine happens once in the epilogue.
            w = C // nch
            ssum_parts = small.tile([P, nch * T], f32)
            for _rep in range(repeat):
                for t in range(T):
                    for h in range(nch):
                        xc = big.tile([P, w], f32, tag="xc")
                        nc.sync.dma_start(
                            out=xc[:], in_=costh[t * P:(t + 1) * P, h * w:(h + 1) * w]
                        )
                        nc.scalar.activation(
                            out=xc[:],
                            in_=xc[:],
                            func=Act.Exp,
                            bias=neg_sm[:],
                            scale=S,
                            accum_out=ssum_parts[:, h * T + t:h * T + t + 1],
                        )

            # --- tail: ssums = sum_h parts; z = ssums - e12;
            #     loss_dev = ln(z) - S*ct_adj ---
            ssums = small.tile([P, T], f32)
            nc.vector.tensor_add(
                out=ssums[:], in0=ssum_parts[:, 0:T], in1=ssum_parts[:, T:2 * T]
            )
            for h in range(2, nch):
                nc.vector.tensor_add(
                    out=ssums[:], in0=ssums[:],
                    in1=ssum_parts[:, h * T:(h + 1) * T],
                )
            z = small.tile([P, T], f32)
            nc.vector.tensor_sub(out=z[:], in0=ssums[:], in1=e12[:])
            lnz = small.tile([P, T], f32)
            nc.scalar.activation(out=lnz[:], in_=z[:], func=Act.Ln)
            loss = small.tile([P, T], f32)
            nc.vector.scalar_tensor_tensor(
                out=loss[:], in0=ct_adj[:], scalar=-S, in1=lnz[:],
                op0=Alu.mult, op1=Alu.add,
            )
            nc.sync.dma_start(out=out[:], in_=loss[:])

    nc.compile()
    return nc


def _get_nc():
    if "nc" not in _NC_CACHE:
        _NC_CACHE["nc"] = _build_nc()
    return _NC_CACHE["nc"]


def _run(costh_np, label_np, trace=False, **spmd_kwargs):
    from concourse.bass_utils import run_bass_kernel_spmd

    nc = _get_nc()
    costh_np = np.ascontiguousarray(costh_np, dtype=np.float32)
    label_i32 = np.ascontiguousarray(label_np).astype(np.int32)
    in_maps = [
        {
            "costh": costh_np[k * R:(k + 1) * R],
            "label": label_i32[k * R:(k + 1) * R],
        }
        for k in range(NCORES)
    ]
    # The first execution of a fresh NEFF through the axon tunnel
    # occasionally faults with NRT_EXEC_UNIT_UNRECOVERABLE; failures are
    # loud (exception, never silent corruption), so a bounded retry is safe.
    # A non-finite total also triggers a retry as extra insurance.
    last_exc = None
    for _attempt in range(3):
        try:
            res = run_bass_kernel_spmd(
                nc, in_maps, core_ids=list(range(NCORES)), trace=trace,
                **spmd_kwargs
            )
            total = sum(r["out"].astype(np.float64).sum() for r in res.results)
            if np.isfinite(total):
                break
            last_exc = RuntimeError("non-finite loss from device")
        except Exception as exc:  # noqa: BLE001
            last_exc = exc
    else:
        raise last_exc
    loss = np.float32(total / B + S * MAXC)
    return loss, res


def kernel(costh, label):
    loss, _ = _run(costh, label)
    return loss


# revision 4
# speedup vs baseline: 165.8764x; 1.0242x over previous
"""DAM-Softmax loss kernel for Trainium2 (Bass/Tile), 8-core data parallel.

Math (per sample b, target t = label[b]):
    cos_t  = costh[b, t]
    delta  = (MARGIN/LAMDA) * exp(1 - cos_t)
    logits = S * costh, with logits[b, t] = S * (cos_t - delta)
    loss_b = logsumexp(logits[b, :]) - S * (cos_t - delta)
    loss   = mean_b loss_b

Since costh is bounded in [0, 1), we use the constant M = 1.0 as the
stability shift (exp arguments stay in [-S, 0]) instead of a per-row max:
    ssum   = sum_j exp(S*(costh[b,j] - M))
    Z      = ssum - exp(S*(cos_t - M)) + exp(S*(cos_t - delta - M))
    loss_b = S*M + ln(Z) - S*(cos_t - delta)

Sharding: batch dim split evenly across 8 NeuronCores (data parallel).
Each core streams its [1024, 10000] f32 shard from HBM once (memory-bound),
computes per-sample losses on device, and the host mean-reduces the
8 x [128, 8] per-sample loss outputs.

Tuning (measured via the wall-clock slope method, see timing.py):
  - DMA shape: 2 column chunks per [128, 10000] row tile -> 2.56 MB
    transfers with 20 kB contiguous per partition; sustains ~429 GB/s/core
    vs ~389 GB/s at the previous 8-chunk shape.
  - bufs=4 on the streaming pool (deeper made it slower).
  - The per-row exp-sum partials accumulate via ACT accum_out only; the
    DVE reduce over partials runs ONCE in the epilogue instead of per row
    tile (the in-loop reduce cost ~10 us/pass of DMA/ACT interference).
  - Partials are laid out chunk-major [P, NCH*T] so the epilogue combine
    is one contiguous tensor_add, not a strided gather.
"""

import numpy as np

NCORES = 8
B, C = 8192, 10000
R = B // NCORES          # rows per core
P = 128                  # SBUF partitions
T = R // P               # row tiles per core
NCH = 2                  # column chunks per row tile
W = C // NCH             # chunk width
S = 15.0
MARGIN = 0.3
LAMDA = 2.0
DCOEF = MARGIN / LAMDA
MAXC = 1.0               # upper bound of costh (uniform [0,1)) used as exp shift

_NC_CACHE = {}


def _build_nc(big_bufs=4, repeat=1, nch=NCH):
    # repeat > 1 re-streams the shard `repeat` times inside one NEFF; used by
    # the timing harness to infer per-pass device time from the wall-clock
    # slope (axon dispatch overhead cancels in the difference).
    import concourse.bacc as bacc
    import concourse.bass as bass
    import concourse.mybir as mybir
    import concourse.tile as tile

    f32 = mybir.dt.float32
    i32 = mybir.dt.int32
    Act = mybir.ActivationFunctionType
    Alu = mybir.AluOpType

    nc = bacc.Bacc(None, target_bir_lowering=False, debug=False)

    costh = nc.dram_tensor("costh", [R, C], f32, kind="ExternalInput")
    label = nc.dram_tensor("label", [R], i32, kind="ExternalInput")
    out = nc.dram_tensor("out", [P, T], f32, kind="ExternalOutput")

    with tile.TileContext(nc) as tc:
        with (
            tc.tile_pool(name="big", bufs=big_bufs) as big,
            tc.tile_pool(name="small", bufs=1) as small,
        ):
            # bias vector for exp(S*x - S*M) activations
            neg_sm = small.tile([P, 1], f32)
            nc.vector.memset(neg_sm[:], -S * MAXC)

            # --- prologue: gather target cosines cos_t[p, t] = costh[t*P+p, label] ---
            label_sb = small.tile([P, T], i32)
            nc.gpsimd.dma_start(
                out=label_sb[:], in_=label[:].rearrange("(t p) -> p t", p=P)
            )
            # idx[p, t] = (t*P + p) * C + label  (flat element index), computed
            # in f32 (exact: values < 2^24) since iota steps are limited to i16.
            row_i = small.tile([P, T], i32)
            nc.gpsimd.iota(row_i[:], pattern=[[P, T]], base=0, channel_multiplier=1)
            row_f = small.tile([P, T], f32)
            nc.vector.tensor_copy(out=row_f[:], in_=row_i[:])
            lab_f = small.tile([P, T], f32)
            nc.vector.tensor_copy(out=lab_f[:], in_=label_sb[:])
            idx_f = small.tile([P, T], f32)
            nc.vector.scalar_tensor_tensor(
                out=idx_f[:], in0=row_f[:], scalar=float(C), in1=lab_f[:],
                op0=Alu.mult, op1=Alu.add,
            )
            idx = small.tile([P, T], i32)
            nc.vector.tensor_copy(out=idx[:], in_=idx_f[:])
            # one indirect DMA per column: HW honors only one index per
            # partition per gather (multi-column offset APs misbehave on HW)
            cos_t = small.tile([P, T], f32)
            for t in range(T):
                nc.gpsimd.indirect_dma_start(
                    out=cos_t[:, t:t + 1],
                    out_offset=None,
                    in_=costh[:, :],
                    in_offset=bass.IndirectOffsetOnAxis(ap=idx[:, t:t + 1], axis=1),
                )

            # target-term math depends only on cos_t, so it is emitted before
            # the stream and overlaps it:
            #   delta_e = exp(1 - cos_t);  ct_adj = cos_t - DCOEF * delta_e
            #   e12 = exp(S*(cos_t - M)) - exp(S*(ct_adj - M))
            delta_e = small.tile([P, T], f32)
            nc.scalar.activation(
                out=delta_e[:], in_=cos_t[:], func=Act.Exp, bias=1.0, scale=-1.0
            )
            ct_adj = small.tile([P, T], f32)
            nc.vector.scalar_tensor_tensor(
                out=ct_adj[:], in0=delta_e[:], scalar=-DCOEF, in1=cos_t[:],
                op0=Alu.mult, op1=Alu.add,
            )
            e1 = small.tile([P, T], f32)
            nc.scalar.activation(
                out=e1[:], in_=cos_t[:], func=Act.Exp, bias=neg_sm[:], scale=S
            )
            e2 = small.tile([P, T], f32)
            nc.scalar.activation(
                out=e2[:], in_=ct_adj[:], func=Act.Exp, bias=neg_sm[:], scale=S
            )
            e12 = small.tile([P, T], f32)
            nc.vector.tensor_sub(out=e12[:], in0=e1[:], in1=e2[:])

            # --- main loop: stream shard, fused exp + row-sum on ACT ---
            # partials land chunk-major: column h*T + t, so chunk h of all
            # row tiles is one contiguous [P, T] block. No DVE work in the
            # loop — the combine happens once in the epilogue.
            #
            # The very last row tile (final repeat only) streams in LAST_NCH
            # smaller chunks instead of nch: the final chunk's ACT is on the
            # post-stream critical path, and a [P, C/LAST_NCH] exp clears in
            # ~1 us instead of ~4 us.
            w = C // nch
            last_nch = max(nch, 8)
            wl = C // last_nch
            ssum_parts = small.tile([P, nch * T], f32)
            parts_last = small.tile([P, last_nch], f32)
            # at repeat=1 the split path leaves ssum_parts' T-1 columns
            # unwritten; zero them so the epilogue's full-width add is legal
            nc.vector.memset(ssum_parts[:], 0.0)
            for _rep in range(repeat):
                final = _rep == repeat - 1
                for t in range(T):
                    if final and t == T - 1:
                        for h in range(last_nch):
                            xl = big.tile([P, wl], f32, tag="xl")
                            nc.sync.dma_start(
                                out=xl[:],
                                in_=costh[t * P:(t + 1) * P, h * wl:(h + 1) * wl],
                            )
                            nc.scalar.activation(
                                out=xl[:], in_=xl[:], func=Act.Exp,
                                bias=neg_sm[:], scale=S,
                                accum_out=parts_last[:, h:h + 1],
                            )
                        continue
                    for h in range(nch):
                        xc = big.tile([P, w], f32, tag="xc")
                        nc.sync.dma_start(
                            out=xc[:], in_=costh[t * P:(t + 1) * P, h * w:(h + 1) * w]
                        )
                        nc.scalar.activation(
                            out=xc[:],
                            in_=xc[:],
                            func=Act.Exp,
                            bias=neg_sm[:],
                            scale=S,
                            accum_out=ssum_parts[:, h * T + t:h * T + t + 1],
                        )

            # --- tail: ssums = sum_h parts; z = ssums - e12;
            #     loss_dev = ln(z) - S*ct_adj ---
            # ssums[:, T-1] from the chunk-major partials is stale (the last
            # tile's exp-sums live in parts_last); the reduce_sum overwrite
            # below supplies the real value.
            ssums = small.tile([P, T], f32)
            nc.vector.tensor_add(
                out=ssums[:], in0=ssum_parts[:, 0:T], in1=ssum_parts[:, T:2 * T]
            )
            for h in range(2, nch):
                nc.vector.tensor_add(
                    out=ssums[:], in0=ssums[:],
                    in1=ssum_parts[:, h * T:(h + 1) * T],
                )
            nc.vector.reduce_sum(
                out=ssums[:, T - 1:T], in_=parts_last[:],
                axis=mybir.AxisListType.X,
            )
            z = small.tile([P, T], f32)
            nc.vector.tensor_sub(out=z[:], in0=ssums[:], in1=e12[:])
            lnz = small.tile([P, T], f32)
            nc.scalar.activation(out=lnz[:], in_=z[:], func=Act.Ln)
            loss = small.tile([P, T], f32)
            nc.vector.scalar_tensor_tensor(
                out=loss[:], in0=ct_adj[:], scalar=-S, in1=lnz[:],
                op0=Alu.mult, op1=Alu.add,
            )
            nc.sync.dma_start(out=out[:], in_=loss[:])

    nc.compile()
    return nc


def _get_nc():
    if "nc" not in _NC_CACHE:
        _NC_CACHE["nc"] = _build_nc()
    return _NC_CACHE["nc"]


def _run(costh_np, label_np, trace=False, **spmd_kwargs):
    from concourse.bass_utils import run_bass_kernel_spmd

    nc = _get_nc()
    costh_np = np.ascontiguousarray(costh_np, dtype=np.float32)
    label_i32 = np.ascontiguousarray(label_np).astype(np.int32)
    in_maps = [
        {
            "costh": costh_np[k * R:(k + 1) * R],
            "label": label_i32[k * R:(k + 1) * R],
        }
        for k in range(NCORES)
    ]
    # The first execution of a fresh NEFF through the axon tunnel
    # occasionally faults with NRT_EXEC_UNIT_UNRECOVERABLE; failures are
    # loud (exception, never silent corruption), so a bounded retry is safe.
    # A non-finite total also triggers a retry as extra insurance.
    last_exc = None
    for _attempt in range(3):
        try:
            res = run_bass_kernel_spmd(
                nc, in_maps, core_ids=list(range(NCORES)), trace=trace,
                **spmd_kwargs
            )
            total = sum(r["out"].astype(np.float64).sum() for r in res.results)
            if np.isfinite(total):
                break
            last_exc = RuntimeError("non-finite loss from device")
        except Exception as exc:  # noqa: BLE001
            last_exc = exc
    else:
        raise last_exc
    loss = np.float32(total / B + S * MAXC)
    return loss, res


def kernel(costh, label):
    loss, _ = _run(costh, label)
    return loss
